# revision 44
# baseline (speedup 1.0000x reference)
"""Trainium2 Bass kernel for FlowNet-style Correlation (MAX_DISP=4).

Input:  x_1, x_2  [8, 64, 256, 256] f32
Output:           [8, 81, 256, 256] f32
out[b, 9*dx+dy, h, w] = mean_c x1[b,c,h,w] * x2pad[b,c,h+dx,w+dy]
(x2pad zero-padded by 4 on each spatial side)

Strategy (data-parallel, 1 image per NeuronCore, 8 cores):
- Inputs stream in as bf16 via casting gpsimd (SWDGE) DMAs -- fp32 HBM reads,
  bf16 SBUF tiles -- so the PE runs bf16 matmuls at 1 cycle/moving-column
  (4x the fp32 rate) with no on-chip cast traffic.
- Vertical-strip band matmuls: stationary = x1 strip [C=64, 32 pixels] (one w
  column, 32 rows of h), moving = x2 window [64, 40 rows x 9 dy = 360].
  psum[32s+i, 9r+dy] = sum_c x1[c,h0+i,w] * x2pad[c,h0+r-4,w+dy-4].
  Useful values for pixel i are the contiguous run psum[p, 9i : 9i+81]
  (a per-partition diagonal -> not extractable on-chip with uniform APs).
- Scale by 1/64 while copying psum->SBUF (bf16) into a staging tile covering
  16 strip-groups, then ONE 3-dim DMA dumps the whole batch to internal DRAM:
  the target layout addr = q*(32N) + i*N + n (q = w-column = 4*grp+s) is
  linear in the raw psum row p = 32s+i since q*32N + i*N = grp*128N + p*N.
- Diagonal re-read (legal: DRAM-side strides are arbitrary): dg[q, (i,d)]
  from addr q*32N + i*(N+9) + d, 81-element contiguous bf16 runs, one DMA
  per 32 pixel-rows per w-half.
- bf16 PE-transpose (1 cycle/row) to [81, 128]; the copy out of PSUM is the
  free bf16->fp32 upcast. Assemble 16 output rows [81, 16*128] per out DMA
  (512B runs, d-major).
- Engine placement matters in the cost model (a DMA occupies its issuing
  engine for its full span): loads+dumps mostly on Pool/ACT, diag+out on SP,
  psum copies split DVE/ACT, keeping every engine under the PE's ~47us/band.
- Software pipeline: extraction of band b-1 interleaves between the two
  compute halves of band b (per-half dump tensors keep the dependencies
  half-granular); input loads prefetch one band ahead; the last band's own
  extraction interleaves into its second compute half.
"""
import os
import numpy as np
from contextlib import ExitStack

import concourse.bass as bass
import concourse.tile as tile
from concourse import bacc, mybir
from concourse.bass_utils import run_bass_kernel_spmd

F32 = mybir.dt.float32
BF16 = mybir.dt.bfloat16

B, C, H, W = 8, 64, 256, 256
MD = 4
D = 2 * MD + 1        # 9
DD = D * D            # 81
BAND = 32             # strip height (pixels per matmul column group)
NBANDS = H // BAND    # 8
WIN_R = BAND + 2 * MD  # 40 window rows
N = WIN_R * D         # 360 moving columns / psum width
NGRP = W // 4         # 64 groups of 4 strips per band
GB = 16               # strip-groups batched per dump DMA
KI = 32               # pixel-rows per diagonal re-read DMA
RO = 16               # output rows assembled per out DMA
INV_C = 1.0 / C


def build_kernel(nbands=NBANDS):
    nc = bacc.Bacc("TRN2", target_bir_lowering=False, debug=False, num_swdge_queues=4)
    x1 = nc.dram_tensor("x1", [C, H, W], F32, kind="ExternalInput").ap()
    x2 = nc.dram_tensor("x2", [C, H, W], F32, kind="ExternalInput").ap()
    identb = nc.dram_tensor("identb", [128, 128], BF16, kind="ExternalInput").ap()
    out = nc.dram_tensor("out", [DD, H, W], F32, kind="ExternalOutput").ap()

    with tile.TileContext(nc) as tc, ExitStack() as ctx:
        xpool = ctx.enter_context(tc.tile_pool(name="xpool", bufs=2))
        spool = ctx.enter_context(tc.tile_pool(name="spool", bufs=3))
        dpool = ctx.enter_context(tc.tile_pool(name="dpool", bufs=3))
        tpool = ctx.enter_context(tc.tile_pool(name="tpool", bufs=3))
        cpool = ctx.enter_context(tc.tile_pool(name="cpool", bufs=1))
        pspool = ctx.enter_context(tc.tile_pool(name="pspool", bufs=5, space="PSUM"))
        tpspool = ctx.enter_context(tc.tile_pool(name="tpspool", bufs=3, space="PSUM"))
        drampool = ctx.enter_context(tc.tile_pool(name="drampool", bufs=2, space="DRAM"))

        sb_ident = cpool.tile([128, 128], BF16)
        nc.sync.dma_start(out=sb_ident[:], in_=identb)

        def load_x1(band):
            h0 = band * BAND
            # gpsimd (SWDGE) DMAs cast in flight: fp32 HBM -> bf16 SBUF
            x1b = xpool.tile([C, BAND, W], BF16, tag="x1b")
            nc.gpsimd.dma_start(out=x1b[:], in_=x1[:, h0 : h0 + BAND, :])
            return x1b

        def load_x2(band):
            h0 = band * BAND
            r0, r1 = 0, WIN_R
            x2b = xpool.tile([C, WIN_R, W + 2 * MD], BF16, tag="x2b")
            nc.vector.memset(x2b[:, :, 0:MD], 0.0)
            nc.vector.memset(x2b[:, :, W + MD : W + 2 * MD], 0.0)
            if band == 0:
                r0 = MD
                nc.vector.memset(x2b[:, 0:MD, :], 0.0)
            if band == nbands - 1:
                r1 = WIN_R - MD
                nc.vector.memset(x2b[:, WIN_R - MD : WIN_R, :], 0.0)
            nc.gpsimd.dma_start(
                out=x2b[:, r0:r1, MD : MD + W],
                in_=x2[:, h0 - MD + r0 : h0 - MD + r1, :],
            )
            return x2b

        def load_band(band):
            return load_x1(band), load_x2(band)

        def make_dumps(band):
            # dump layout: element (q=w-column, i, n) at q*32*N + i*N + n, so
            # the diagonal re-read's 128 partitions (q) use a single stride.
            # Split left/right half so extraction of a half only waits on
            # that half's dumps.
            dumps = []
            for hf in range(2):
                dump_h = drampool.tile(
                    [128, 32 * N], BF16, tag=f"dump{hf}", name=f"dump{hf}_{band}"
                )
                dumps.append(dump_h)
            return dumps

        def compute_half(band, tiles, dumps, hf, mid=None):
            """matmuls + batched bf16 dump for one half (128 w) of a band."""
            x1b, x2b = tiles
            nper = NGRP // GB // 2            # g8 batches per half
            for g8 in range(hf * nper, (hf + 1) * nper):
                if mid is not None and g8 != hf * nper:
                    mid()
                    mid = None
                stg = spool.tile([128, GB * N], BF16, tag="stg")
                for gg in range(GB):
                    grp = g8 * GB + gg
                    ps = pspool.tile([128, 512], F32, tag="ps")  # bank-aligned
                    for s in range(4):
                        w = 4 * grp + s
                        nc.tensor.matmul(
                            ps[32 * s : 32 * (s + 1), 0:N],
                            x1b[:, :, w],                  # [64, 32] stationary
                            x2b[:, :, w : w + D],          # [64, 40, 9] moving
                            start=True,
                            stop=True,
                            tile_position=(0, 32 * s),
                        )
                    dst = stg[:, N * gg : N * (gg + 1)]
                    # copies split 2:1 DVE:ACT (ACT also issues a dump DMA)
                    if grp % 3 != 0:
                        nc.vector.tensor_scalar_mul(dst, ps[:, 0:N], INV_C)
                    else:
                        nc.scalar.activation(
                            dst, ps[:, 0:N], mybir.ActivationFunctionType.Copy,
                            scale=INV_C,
                        )
                # dump layout addr = q*32N + i*N + n with q = 4*grp+s is
                # LINEAR in the raw psum row p = 32s+i:
                #   q*32N + i*N = 128N*grp + N*(32s+i) = grp*128N + p*N
                # so the whole stg batch goes out in ONE 3-dim DMA:
                #   [[N, 128](p), [128N, GB](gg), [1, N](n)]
                # Alternate Pool (SWDGE, otherwise idle) and ACT so no single
                # engine's DMA pipeline paces the band.
                dump_dst = bass.AP(
                    tensor=dumps[hf].tensor,
                    offset=dumps[hf].offset + (GB * g8 - hf * 32) * (128 * N),
                    ap=[[N, 128], [128 * N, GB], [1, N]],
                )
                deng = nc.scalar if g8 % 4 == 3 else nc.gpsimd
                deng.dma_start(out=dump_dst, in_=stg[:])

        def extract_half(band, dumps, half):
            """diagonal re-read (KI rows per DMA) + transpose + store, for
            one w-half (128 columns) of a band."""
            h0 = band * BAND
            for i8 in range(BAND // KI):
                i0 = KI * i8
                dg = dpool.tile([128, KI * DD], BF16, tag="dg")
                # element (q, i, d) at q*32*N + i*N + 9i + d; i advances
                # both the row (N) and the in-row offset (9) -> N+9.
                diag_src = bass.AP(
                    tensor=dumps[half].tensor,
                    offset=dumps[half].offset + i0 * (N + D),
                    ap=[[32 * N, 128], [N + D, KI], [1, DD]],
                )
                dge = nc.sync if (band + half) % 2 == 0 else nc.gpsimd
                dge.dma_start(out=dg[:], in_=diag_src)
                for r4 in range(KI // RO):
                    trans = tpool.tile([DD, RO * 128], F32, tag="trans")
                    for t in range(RO):
                        tt = RO * r4 + t
                        tps = tpspool.tile([DD, 128], BF16, tag="tps")
                        nc.tensor.transpose(
                            tps[:], dg[:, DD * tt : DD * (tt + 1)], sb_ident[:]
                        )
                        cdst = trans[:, 128 * t : 128 * (t + 1)]
                        if t % 2 == 0:
                            nc.vector.tensor_copy(cdst, tps[:])
                        else:
                            nc.scalar.activation(
                                cdst, tps[:], mybir.ActivationFunctionType.Copy
                            )
                    i_first = i0 + RO * r4
                    out_dst = bass.AP(
                        tensor=out.tensor,
                        offset=out.offset + (h0 + i_first) * W + half * 128,
                        ap=[[H * W, DD], [W, RO], [1, 128]],
                    )
                    oeng = nc.sync if (r4 + half) % 2 == 0 else nc.gpsimd
                    oeng.dma_start(out=out_dst, in_=trans[:])

        # software pipeline: extraction of band b-1 interleaves with the two
        # compute halves of band b; loads prefetch one band ahead.
        tiles = load_band(0)
        prev_dumps = None
        for band in range(nbands):
            dumps = make_dumps(band)
            nx1 = nx2 = None
            compute_half(band, tiles, dumps, 0)
            if band + 1 < nbands:
                nx1 = load_x1(band + 1)
            if prev_dumps is not None:
                extract_half(band - 1, prev_dumps, 0)
            if band + 1 < nbands:
                nx2 = load_x2(band + 1)
            last = band == nbands - 1
            compute_half(
                band, tiles, dumps, 1,
                mid=(lambda: extract_half(band, dumps, 0)) if last else None,
            )
            if prev_dumps is not None:
                extract_half(band - 1, prev_dumps, 1)
            if band + 1 < nbands:
                tiles = (nx1, nx2)
            prev_dumps = dumps
        extract_half(nbands - 1, prev_dumps, 1)

    nc.compile()
    return nc


_NC_CACHE = {}


def _get_nc():
    if "nc" not in _NC_CACHE:
        _NC_CACHE["nc"] = build_kernel()
    return _NC_CACHE["nc"]


def kernel(x_1: np.ndarray, x_2: np.ndarray) -> np.ndarray:
    import ml_dtypes

    x_1 = np.asarray(x_1, dtype=np.float32)
    x_2 = np.asarray(x_2, dtype=np.float32)
    nc = _get_nc()
    eye16 = np.eye(128, dtype=ml_dtypes.bfloat16)
    in_maps = [
        {"x1": np.ascontiguousarray(x_1[b]), "x2": np.ascontiguousarray(x_2[b]),
         "identb": eye16}
        for b in range(B)
    ]
    res = run_bass_kernel_spmd(
        nc, in_maps, core_ids=list(range(B)),
        trace=bool(int(os.environ.get("CORR_TRACE", "0"))),
    )
    out = np.stack([res.results[b]["out"] for b in range(B)], axis=0)
    if int(os.environ.get("CORR_TRACE", "0")):
        _NC_CACHE["last_results"] = res
    return out


# revision 48
# speedup vs baseline: 1.0021x; 1.0021x over previous
"""Trainium2 Bass kernel for FlowNet-style Correlation (MAX_DISP=4).

Input:  x_1, x_2  [8, 64, 256, 256] f32
Output:           [8, 81, 256, 256] f32
out[b, 9*dx+dy, h, w] = mean_c x1[b,c,h,w] * x2pad[b,c,h+dx,w+dy]
(x2pad zero-padded by 4 on each spatial side)

Strategy (data-parallel, 1 image per NeuronCore, 8 cores):
- Inputs stream in as bf16 via casting gpsimd (SWDGE) DMAs -- fp32 HBM reads,
  bf16 SBUF tiles -- so the PE runs bf16 matmuls at 1 cycle/moving-column
  (4x the fp32 rate) with no on-chip cast traffic.
- Vertical-strip band matmuls: stationary = x1 strip [C=64, 32 pixels] (one w
  column, 32 rows of h), moving = x2 window [64, 40 rows x 9 dy = 360].
  psum[32s+i, 9r+dy] = sum_c x1[c,h0+i,w] * x2pad[c,h0+r-4,w+dy-4].
  Useful values for pixel i are the contiguous run psum[p, 9i : 9i+81]
  (a per-partition diagonal -> not extractable on-chip with uniform APs).
- Scale by 1/64 while copying psum->SBUF (bf16) into a staging tile covering
  16 strip-groups, then ONE 3-dim DMA dumps the whole batch to internal DRAM:
  the target layout addr = q*(32N) + i*N + n (q = w-column = 4*grp+s) is
  linear in the raw psum row p = 32s+i since q*32N + i*N = grp*128N + p*N.
- Diagonal re-read (legal: DRAM-side strides are arbitrary): dg[q, (i,d)]
  from addr q*32N + i*(N+9) + d, 81-element contiguous bf16 runs, one DMA
  per 32 pixel-rows per w-half.
- bf16 PE-transpose (1 cycle/row) to [81, 128]; the copy out of PSUM is the
  free bf16->fp32 upcast. Assemble 16 output rows [81, 16*128] per out DMA
  (512B runs, d-major).
- Engine placement matters in the cost model (a DMA occupies its issuing
  engine for its full span): loads+dumps mostly on Pool/ACT, diag+out on SP,
  psum copies split DVE/ACT, keeping every engine under the PE's ~47us/band.
- Software pipeline: extraction of band b-1 interleaves between the two
  compute halves of band b (per-half dump tensors keep the dependencies
  half-granular); input loads prefetch one band ahead; the last band's own
  extraction interleaves into its second compute half.
"""
import os
import numpy as np
from contextlib import ExitStack

import concourse.bass as bass
import concourse.tile as tile
from concourse import bacc, mybir
from concourse.bass_utils import run_bass_kernel_spmd

F32 = mybir.dt.float32
BF16 = mybir.dt.bfloat16

B, C, H, W = 8, 64, 256, 256
MD = 4
D = 2 * MD + 1        # 9
DD = D * D            # 81
BAND = 32             # strip height (pixels per matmul column group)
NBANDS = H // BAND    # 8
WIN_R = BAND + 2 * MD  # 40 window rows
N = WIN_R * D         # 360 moving columns / psum width
NGRP = W // 4         # 64 groups of 4 strips per band
GB = 16               # strip-groups batched per dump DMA
KI = 32               # pixel-rows per diagonal re-read DMA
RO = 16               # output rows assembled per out DMA
INV_C = 1.0 / C


def build_kernel(nbands=NBANDS):
    nc = bacc.Bacc("TRN2", target_bir_lowering=False, debug=False, num_swdge_queues=4)
    x1 = nc.dram_tensor("x1", [C, H, W], F32, kind="ExternalInput").ap()
    x2 = nc.dram_tensor("x2", [C, H, W], F32, kind="ExternalInput").ap()
    identb = nc.dram_tensor("identb", [128, 128], BF16, kind="ExternalInput").ap()
    out = nc.dram_tensor("out", [DD, H, W], F32, kind="ExternalOutput").ap()

    with tile.TileContext(nc) as tc, ExitStack() as ctx:
        xpool = ctx.enter_context(tc.tile_pool(name="xpool", bufs=2))
        spool = ctx.enter_context(tc.tile_pool(name="spool", bufs=3))
        dpool = ctx.enter_context(tc.tile_pool(name="dpool", bufs=3))
        tpool = ctx.enter_context(tc.tile_pool(name="tpool", bufs=3))
        cpool = ctx.enter_context(tc.tile_pool(name="cpool", bufs=1))
        pspool = ctx.enter_context(tc.tile_pool(name="pspool", bufs=5, space="PSUM"))
        tpspool = ctx.enter_context(tc.tile_pool(name="tpspool", bufs=3, space="PSUM"))
        drampool = ctx.enter_context(tc.tile_pool(name="drampool", bufs=2, space="DRAM"))

        sb_ident = cpool.tile([128, 128], BF16)
        nc.sync.dma_start(out=sb_ident[:], in_=identb)

        def load_x1(band):
            h0 = band * BAND
            # gpsimd (SWDGE) DMAs cast in flight: fp32 HBM -> bf16 SBUF
            x1b = xpool.tile([C, BAND, W], BF16, tag="x1b")
            nc.gpsimd.dma_start(out=x1b[:], in_=x1[:, h0 : h0 + BAND, :])
            return x1b

        def load_x2(band):
            h0 = band * BAND
            r0, r1 = 0, WIN_R
            x2b = xpool.tile([C, WIN_R, W + 2 * MD], BF16, tag="x2b")
            nc.vector.memset(x2b[:, :, 0:MD], 0.0)
            nc.vector.memset(x2b[:, :, W + MD : W + 2 * MD], 0.0)
            if band == 0:
                r0 = MD
                nc.vector.memset(x2b[:, 0:MD, :], 0.0)
            if band == nbands - 1:
                r1 = WIN_R - MD
                nc.vector.memset(x2b[:, WIN_R - MD : WIN_R, :], 0.0)
            nc.gpsimd.dma_start(
                out=x2b[:, r0:r1, MD : MD + W],
                in_=x2[:, h0 - MD + r0 : h0 - MD + r1, :],
            )
            return x2b

        def load_band(band):
            return load_x1(band), load_x2(band)

        def make_dumps(band):
            # dump layout: element (q=w-column, i, n) at q*32*N + i*N + n, so
            # the diagonal re-read's 128 partitions (q) use a single stride.
            # Split left/right half so extraction of a half only waits on
            # that half's dumps.
            dumps = []
            for hf in range(2):
                dump_h = drampool.tile(
                    [128, 32 * N], BF16, tag=f"dump{hf}", name=f"dump{hf}_{band}"
                )
                dumps.append(dump_h)
            return dumps

        def compute_half(band, tiles, dumps, hf, mid=None):
            """matmuls + batched bf16 dump for one half (128 w) of a band."""
            x1b, x2b = tiles
            nper = NGRP // GB // 2            # g8 batches per half
            for g8 in range(hf * nper, (hf + 1) * nper):
                if mid is not None and g8 != hf * nper:
                    mid()
                    mid = None
                stg = spool.tile([128, GB * N], BF16, tag="stg")
                for gg in range(GB):
                    grp = g8 * GB + gg
                    ps = pspool.tile([128, 512], F32, tag="ps")  # bank-aligned
                    for s in range(4):
                        w = 4 * grp + s
                        nc.tensor.matmul(
                            ps[32 * s : 32 * (s + 1), 0:N],
                            x1b[:, :, w],                  # [64, 32] stationary
                            x2b[:, :, w : w + D],          # [64, 40, 9] moving
                            start=True,
                            stop=True,
                            tile_position=(0, 32 * s),
                        )
                    dst = stg[:, N * gg : N * (gg + 1)]
                    # copies split 2:1 DVE:ACT (ACT also issues a dump DMA)
                    if grp % 3 != 0:
                        nc.vector.tensor_scalar_mul(dst, ps[:, 0:N], INV_C)
                    else:
                        nc.scalar.activation(
                            dst, ps[:, 0:N], mybir.ActivationFunctionType.Copy,
                            scale=INV_C,
                        )
                # dump layout addr = q*32N + i*N + n with q = 4*grp+s is
                # LINEAR in the raw psum row p = 32s+i:
                #   q*32N + i*N = 128N*grp + N*(32s+i) = grp*128N + p*N
                # so the whole stg batch goes out in ONE 3-dim DMA:
                #   [[N, 128](p), [128N, GB](gg), [1, N](n)]
                # Alternate Pool (SWDGE, otherwise idle) and ACT so no single
                # engine's DMA pipeline paces the band.
                dump_dst = bass.AP(
                    tensor=dumps[hf].tensor,
                    offset=dumps[hf].offset + (GB * g8 - hf * 32) * (128 * N),
                    ap=[[N, 128], [128 * N, GB], [1, N]],
                )
                # ACT takes 1-in-4 dumps except on the last band, where its
                # transpose-copy queue must stay clear for the final extraction
                last_band = band == nbands - 1
                deng = nc.scalar if (g8 % 4 == 3 and not last_band) else nc.gpsimd
                deng.dma_start(out=dump_dst, in_=stg[:])

        def extract_half(band, dumps, half, ki=KI):
            """diagonal re-read (ki rows per DMA) + transpose + store, for
            one w-half (128 columns) of a band."""
            h0 = band * BAND
            for i8 in range(BAND // ki):
                i0 = ki * i8
                dg = dpool.tile([128, ki * DD], BF16, tag="dg")
                # element (q, i, d) at q*32*N + i*N + 9i + d; i advances
                # both the row (N) and the in-row offset (9) -> N+9.
                diag_src = bass.AP(
                    tensor=dumps[half].tensor,
                    offset=dumps[half].offset + i0 * (N + D),
                    ap=[[32 * N, 128], [N + D, ki], [1, DD]],
                )
                dge = nc.sync if (band + half) % 2 == 0 else nc.gpsimd
                dge.dma_start(out=dg[:], in_=diag_src)
                for r4 in range(max(1, ki // RO)):
                    ro = min(RO, ki)
                    trans = tpool.tile([DD, ro * 128], F32, tag="trans")
                    for t in range(ro):
                        tt = ro * r4 + t
                        tps = tpspool.tile([DD, 128], BF16, tag="tps")
                        nc.tensor.transpose(
                            tps[:], dg[:, DD * tt : DD * (tt + 1)], sb_ident[:]
                        )
                        cdst = trans[:, 128 * t : 128 * (t + 1)]
                        if t % 2 == 0:
                            nc.vector.tensor_copy(cdst, tps[:])
                        else:
                            nc.scalar.activation(
                                cdst, tps[:], mybir.ActivationFunctionType.Copy
                            )
                    i_first = i0 + ro * r4
                    out_dst = bass.AP(
                        tensor=out.tensor,
                        offset=out.offset + (h0 + i_first) * W + half * 128,
                        ap=[[H * W, DD], [W, ro], [1, 128]],
                    )
                    oeng = nc.sync if (r4 + half) % 2 == 0 else nc.gpsimd
                    oeng.dma_start(out=out_dst, in_=trans[:])

        # software pipeline: extraction of band b-1 interleaves with the two
        # compute halves of band b; loads prefetch one band ahead.
        tiles = load_band(0)
        prev_dumps = None
        for band in range(nbands):
            dumps = make_dumps(band)
            nx1 = nx2 = None
            compute_half(band, tiles, dumps, 0)
            if band + 1 < nbands:
                nx1 = load_x1(band + 1)
            if prev_dumps is not None:
                extract_half(band - 1, prev_dumps, 0)
            if band + 1 < nbands:
                nx2 = load_x2(band + 1)
            last = band == nbands - 1
            compute_half(
                band, tiles, dumps, 1,
                mid=(lambda: extract_half(band, dumps, 0, ki=8)) if last else None,
            )
            if prev_dumps is not None:
                extract_half(band - 1, prev_dumps, 1)
            if band + 1 < nbands:
                tiles = (nx1, nx2)
            prev_dumps = dumps
        extract_half(nbands - 1, prev_dumps, 1, ki=8)

    nc.compile()
    return nc


_NC_CACHE = {}


def _get_nc():
    if "nc" not in _NC_CACHE:
        _NC_CACHE["nc"] = build_kernel()
    return _NC_CACHE["nc"]


def kernel(x_1: np.ndarray, x_2: np.ndarray) -> np.ndarray:
    import ml_dtypes

    x_1 = np.asarray(x_1, dtype=np.float32)
    x_2 = np.asarray(x_2, dtype=np.float32)
    nc = _get_nc()
    eye16 = np.eye(128, dtype=ml_dtypes.bfloat16)
    in_maps = [
        {"x1": np.ascontiguousarray(x_1[b]), "x2": np.ascontiguousarray(x_2[b]),
         "identb": eye16}
        for b in range(B)
    ]
    res = run_bass_kernel_spmd(
        nc, in_maps, core_ids=list(range(B)),
        trace=bool(int(os.environ.get("CORR_TRACE", "0"))),
    )
    out = np.stack([res.results[b]["out"] for b in range(B)], axis=0)
    if int(os.environ.get("CORR_TRACE", "0")):
        _NC_CACHE["last_results"] = res
    return out
